# revision 5
# baseline (speedup 1.0000x reference)
"""Trainium2 Bass kernel for nn_BinaryDense: y = x @ binarize(w).T

x: [8192, 4096] f32, weight: [4096, 4096] f32 -> y: [8192, 4096] f32.

binarize(w) = +1 if fp32(w + 1.0) > 1.0 else -1, i.e. w > 2**-24 -> +1
(reference: round-half-even(clip((w+1)/2, 0, 1)) * 2 - 1 with H=1).

Strategy (8 cores):
  - data-parallel over x rows: each core owns a [1024, 4096] shard of x
    and computes its [1024, 4096] slice of y.
  - weight prep is sharded: each core binarizes + transposes only its
    [512, 4096] row shard of w to fp16 {-1,+1} (exact in fp16), then an
    8-core AllGather distributes the binarized transposed weight
    (fp16, 33.5 MB total) to every core.
  - x is cast to fp16 (error ~1e-4 relative on the output, dominated by
    the fp16 rounding of x; the binarized weights are exact) and both
    operands are transposed on the PE (transpose-mode matmuls) so the
    contraction dim sits on SBUF partitions. DMA-xbar transposes are
    deliberately avoided: Tile serializes them against collectives.
  - one fp16 matmul pass per output tile accumulates in fp32 PSUM.
"""

import numpy as np

import concourse.bass as bass
import concourse.tile as tile
from concourse import bacc, mybir
from concourse.bass_utils import run_bass_kernel_spmd
from concourse.masks import make_identity

N_CORES = 8
B = 1024            # rows of x per core
D = 4096            # in/out features
OSH = D // N_CORES  # 512, rows of w per core
BT = 128            # b tile (psum partition)
OT = 512            # o tile (psum free / one bank)
IT = 128            # contraction tile (partitions)
N_BT = B // BT      # 8
N_OT = D // OT      # 8
N_IT = D // IT      # 32
CK = 2048           # free-dim chunk for preprocessing ops
N_HALF = 2          # AllGather pipeline depth (halves of the i range)
IH = D // N_HALF    # 2048 i-rows per collective
XQ = 4              # resident transposed-x split (pipelining granularity)

F32 = mybir.dt.float32
F16 = mybir.dt.float16

BIN_THRESH = float(2.0 ** -24)

_CACHED = {}


def _build():
    nc = bacc.Bacc("TRN2", target_bir_lowering=False, debug=False,
                   num_devices=N_CORES)
    x = nc.dram_tensor("x", [B, D], F32, kind="ExternalInput").ap()
    wsh = nc.dram_tensor("wsh", [OSH, D], F32, kind="ExternalInput").ap()
    y = nc.dram_tensor("y", [B, D], F32, kind="ExternalOutput").ap()
    # binarized transposed shard: [i, o_shard]
    wshT_d = nc.dram_tensor("wshT_d", [D, OSH], F16).ap()
    # AllGather outputs: [core][i_half, o_shard]; block c holds o range
    # [c*512, (c+1)*512) for i rows of that half
    wT_h = [
        nc.dram_tensor(f"wT_h{h}", [N_CORES, IH, OSH], F16,
                       addr_space="Shared").ap()
        for h in range(N_HALF)
    ]

    with tile.TileContext(nc) as tc:
        with (
            tc.tile_pool(name="const", bufs=1) as const,
            tc.tile_pool(name="prep", bufs=3) as prep,
            tc.tile_pool(name="xres", bufs=1) as xres,
            tc.tile_pool(name="wres", bufs=1) as wres,
            tc.tile_pool(name="wmov", bufs=6) as wmov,
            tc.tile_pool(name="drain", bufs=6) as drain,
        ):
            id16 = const.tile([128, 128], F16, tag="id16")
            make_identity(nc, id16[:])
            id32 = const.tile([128, 128], F32, tag="id32")
            make_identity(nc, id32[:])

            # wTs_h[h] holds [128 (i in tile), 16 i-tiles * 512 o]
            wTs_h = [
                wres.tile([128, (N_IT // N_HALF) * OSH], F16, tag=f"wts{h}",
                          name=f"wts{h}")
                for h in range(N_HALF)
            ]
            # resident transposed x: quarter q, slice itq: [128 i, 1024 b]
            xthi = [
                xres.tile([128, (N_IT // XQ) * B], F16, tag=f"xthi{q}",
                          name=f"xthi{q}")
                for q in range(XQ)
            ]

            with tc.tile_pool(name="tpsum", bufs=4, space="PSUM") as tpsum:
                # ---- phase B: w shard -> fp16 {-1,+1}, transposed (PE) ----
                for h in range(N_HALF):           # i halves
                    for rt in range(OSH // 128):  # o row tiles of the shard
                        wa = prep.tile([128, IH], F32, tag="t_f32_a")
                        nc.scalar.dma_start(
                            wa[:], wsh[bass.ts(rt, 128), bass.ts(h, IH)])
                        w01 = prep.tile([128, IH], F32, tag="t_f32_b")
                        nc.vector.tensor_scalar(
                            w01[:], wa[:], BIN_THRESH, None,
                            mybir.AluOpType.is_gt)
                        wb = prep.tile([128, IH], F16, tag="t_f16_a")
                        nc.vector.tensor_scalar(
                            wb[:], w01[:], 2.0, -1.0,
                            mybir.AluOpType.mult, mybir.AluOpType.add)
                        for it in range(IH // 128):
                            tw = tpsum.tile([128, 128], F16, tag="tw")
                            nc.tensor.transpose(
                                tw[:], wb[:, bass.ts(it, 128)], id16[:])
                            nc.vector.tensor_copy(
                                wTs_h[h][:, bass.ds(it * OSH + rt * 128, 128)],
                                tw[:])
                    for it in range(IH // 128):
                        nc.scalar.dma_start(
                            wshT_d[bass.ds(h * IH + it * 128, 128), :],
                            wTs_h[h][:, bass.ts(it, OSH)])

                # ---- AllGather the binarized transposed weight halves ----
                # (issued before phase A so the collectives overlap the
                # x-side prep, which runs on PE/DVE and plain DMAs only)
                for h in range(N_HALF):
                    nc.gpsimd.collective_compute(
                        "AllGather",
                        mybir.AluOpType.bypass,
                        replica_groups=[list(range(N_CORES))],
                        ins=[wshT_d[bass.ts(h, IH), :]],
                        outs=[wT_h[h][:]],
                    )

                # ---- phase A: x -> fp16 transposed resident (PE) ----
                for bt in range(N_BT):
                    for ck in range(D // CK):
                        xa = prep.tile([128, CK], F32, tag="t_f32_a")
                        nc.scalar.dma_start(
                            xa[:], x[bass.ts(bt, BT), bass.ts(ck, CK)])
                        for itl in range(CK // 128):
                            it = ck * (CK // 128) + itl
                            q, itq = divmod(it, N_IT // XQ)
                            tx = tpsum.tile([128, 128], F32, tag="tx")
                            nc.tensor.transpose(
                                tx[:], xa[:, bass.ts(itl, 128)], id32[:])
                            nc.vector.tensor_copy(
                                xthi[q][:, bass.ds(itq * B + bt * BT, BT)],
                                tx[:])

            # ---- phase C: matmul; w streamed once ----
            with tc.tile_pool(name="psum", bufs=8, space="PSUM") as psum:
                for ot in range(N_OT):
                    pts = []
                    for bt in range(N_BT):
                        pt = psum.tile([128, OT], F32, tag="acc")
                        pts.append(pt)
                    for it in range(N_IT):
                        h, ith = divmod(it, N_IT // N_HALF)
                        wt = wmov.tile([128, OT], F16, tag="wmov")
                        nc.sync.dma_start(
                            wt[:], wT_h[h][ot, bass.ts(ith, IT), :])
                        q, itq = divmod(it, N_IT // XQ)
                        for bt in range(N_BT):
                            nc.tensor.matmul(
                                pts[bt][:],
                                xthi[q][:, bass.ds(itq * B + bt * BT, BT)],
                                wt[:],
                                start=(it == 0), stop=(it == N_IT - 1))
                    for bt in range(N_BT):
                        st = drain.tile([128, OT], F32, tag="drain")
                        nc.vector.tensor_copy(st[:], pts[bt][:])
                        nc.scalar.dma_start(
                            y[bass.ts(bt, BT), bass.ts(ot, OT)], st[:])

    nc.finalize()
    return nc


def _get_nc():
    if "nc" not in _CACHED:
        _CACHED["nc"] = _build()
    return _CACHED["nc"]


def run(x, weight, **run_kwargs):
    nc = _get_nc()
    x = np.ascontiguousarray(x, dtype=np.float32)
    weight = np.ascontiguousarray(weight, dtype=np.float32)
    in_maps = [
        {"x": x[c * B:(c + 1) * B], "wsh": weight[c * OSH:(c + 1) * OSH]}
        for c in range(N_CORES)
    ]
    res = run_bass_kernel_spmd(nc, in_maps, list(range(N_CORES)), **run_kwargs)
    out = np.concatenate([res.results[c]["y"] for c in range(N_CORES)], axis=0)
    return out, res


def kernel(x, weight):
    out, _ = run(x, weight)
    return out


# revision 8
# speedup vs baseline: 1.0535x; 1.0535x over previous
"""Trainium2 Bass kernel for nn_BinaryDense: y = x @ binarize(w).T

x: [8192, 4096] f32, weight: [4096, 4096] f32 -> y: [8192, 4096] f32.

binarize(w) = +1 if fp32(w + 1.0) > 1.0 else -1, i.e. w > 2**-24 -> +1
(reference: round-half-even(clip((w+1)/2, 0, 1)) * 2 - 1 with H=1).

Strategy (8 cores):
  - data-parallel over x rows: each core owns a [1024, 4096] shard of x
    and computes its [1024, 4096] slice of y.
  - weight prep is sharded by the contraction dim: core c binarizes +
    transposes w[:, c*512:(c+1)*512] to fp16 {-1,+1} (exact in fp16),
    giving the [512 i, 4096 o] slice of w^T. Two AllGathers (one per
    o-half) distribute it; the matmul phase consumes o-tiles in order,
    so the second AllGather overlaps the first half's matmuls.
  - x is cast to fp16 (error ~1e-4 relative on the output; weights are
    exact) and both operands are transposed on the PE (transpose-mode
    matmuls) so the contraction dim sits on SBUF partitions. DMA-xbar
    transposes are avoided: Tile serializes them against collectives.
  - one fp16 matmul pass per output tile accumulates in fp32 PSUM.
"""

import numpy as np

import concourse.bass as bass
import concourse.tile as tile
from concourse import bacc, mybir
from concourse.bass_utils import run_bass_kernel_spmd
from concourse.masks import make_identity

N_CORES = 8
B = 1024            # rows of x per core
D = 4096            # in/out features
ISH = D // N_CORES  # 512, i-columns of w per core
BT = 128            # b tile (psum partition)
OT = 512            # o tile (psum free / one bank)
IT = 128            # contraction tile (partitions)
N_BT = B // BT      # 8
N_OT = D // OT      # 8
N_IT = D // IT      # 32
CK = 2048           # free-dim chunk for x preprocessing
N_HALF = 2          # AllGather pipeline depth (halves of the o range)
OH = D // N_HALF    # 2048 o-cols per collective
XQ = 4              # resident transposed-x split

F32 = mybir.dt.float32
F16 = mybir.dt.float16

BIN_THRESH = float(2.0 ** -24)

_CACHED = {}


def _build(repeat=1, include_ag=True, include_c=True):
    nc = bacc.Bacc("TRN2", target_bir_lowering=False, debug=False,
                   num_devices=N_CORES)
    x = nc.dram_tensor("x", [B, D], F32, kind="ExternalInput").ap()
    # w columns shard: w[:, c*512:(c+1)*512] -> [4096 o, 512 i]
    wsh = nc.dram_tensor("wsh", [D, ISH], F32, kind="ExternalInput").ap()
    y = nc.dram_tensor("y", [B, D], F32, kind="ExternalOutput").ap()
    # binarized transposed shard, split in o halves: [512 i, 2048 o]
    wshT_o = [
        nc.dram_tensor(f"wshT_o{h}", [ISH, OH], F16).ap()
        for h in range(N_HALF)
    ]
    # AllGather outputs: [core][i_chunk of 512, o half]; block c holds
    # i rows [c*512, (c+1)*512) for o of that half
    wT_o = [
        nc.dram_tensor(f"wT_o{h}", [N_CORES, ISH, OH], F16,
                       addr_space="Shared").ap()
        for h in range(N_HALF)
    ]

    with tile.TileContext(nc) as tc:
      for _rep in range(repeat):
        with (
            tc.tile_pool(name="const", bufs=1) as const,
            tc.tile_pool(name="prep", bufs=3) as prep,
            tc.tile_pool(name="xres", bufs=1) as xres,
            tc.tile_pool(name="wres", bufs=1) as wres,
            tc.tile_pool(name="wmov", bufs=6) as wmov,
            tc.tile_pool(name="drain", bufs=6) as drain,
        ):
            id16 = const.tile([128, 128], F16, tag="id16")
            make_identity(nc, id16[:])
            id32 = const.tile([128, 128], F32, tag="id32")
            make_identity(nc, id32[:])

            # transposed shard staging: slab itl holds [128 i, 4096 o]
            wTs = [
                wres.tile([128, D], F16, tag=f"wts{itl}", name=f"wts{itl}")
                for itl in range(ISH // 128)
            ]
            # resident transposed x: quarter q, slice itq: [128 i, 1024 b]
            xthi = [
                xres.tile([128, (N_IT // XQ) * B], F16, tag=f"xthi{q}",
                          name=f"xthi{q}")
                for q in range(XQ)
            ]

            with tc.tile_pool(name="tpsum", bufs=4, space="PSUM") as tpsum:
                # ---- phase B: w shard -> fp16 {-1,+1}, transposed (PE),
                # emitted in o order so each half's AllGather fires as
                # soon as its o columns are staged ----
                for h in range(N_HALF):
                    for rtl in range(OH // 128):  # o row tiles in half
                        rt = h * (OH // 128) + rtl
                        wa = prep.tile([128, ISH], F32, tag="w_f32_a")
                        nc.scalar.dma_start(
                            wa[:], wsh[bass.ts(rt, 128), :])
                        w01 = prep.tile([128, ISH], F32, tag="w_f32_b")
                        nc.vector.tensor_scalar(
                            w01[:], wa[:], BIN_THRESH, None,
                            mybir.AluOpType.is_gt)
                        wb = prep.tile([128, ISH], F16, tag="w_f16_a")
                        nc.vector.tensor_scalar(
                            wb[:], w01[:], 2.0, -1.0,
                            mybir.AluOpType.mult, mybir.AluOpType.add)
                        for itl in range(ISH // 128):
                            tw = tpsum.tile([128, 128], F16, tag="tw")
                            nc.tensor.transpose(
                                tw[:], wb[:, bass.ts(itl, 128)], id16[:])
                            nc.vector.tensor_copy(
                                wTs[itl][:, bass.ds(rt * 128, 128)], tw[:])
                    # store this half of the transposed shard
                    for itl in range(ISH // 128):
                        nc.scalar.dma_start(
                            wshT_o[h][bass.ts(itl, 128), :],
                            wTs[itl][:, bass.ts(h, OH)])
                    if include_ag:
                        nc.gpsimd.collective_compute(
                            "AllGather",
                            mybir.AluOpType.bypass,
                            replica_groups=[list(range(N_CORES))],
                            ins=[wshT_o[h][:]],
                            outs=[wT_o[h][:]],
                        )

                # ---- phase A: x -> fp16 transposed resident (PE) ----
                for bt in range(N_BT):
                    for ck in range(D // CK):
                        xa = prep.tile([128, CK], F32, tag="x_f32_a")
                        nc.scalar.dma_start(
                            xa[:], x[bass.ts(bt, BT), bass.ts(ck, CK)])
                        for itl in range(CK // 128):
                            it = ck * (CK // 128) + itl
                            q, itq = divmod(it, N_IT // XQ)
                            tx = tpsum.tile([128, 128], F32, tag="tx")
                            nc.tensor.transpose(
                                tx[:], xa[:, bass.ts(itl, 128)], id32[:])
                            nc.vector.tensor_copy(
                                xthi[q][:, bass.ds(itq * B + bt * BT, BT)],
                                tx[:])

            # ---- phase C: matmul; w streamed once ----
            with tc.tile_pool(name="psum", bufs=8, space="PSUM") as psum:
                for ot in range(N_OT if include_c else 0):
                    h, otl = divmod(ot, N_OT // N_HALF)
                    pts = []
                    for bt in range(N_BT):
                        pt = psum.tile([128, OT], F32, tag="acc")
                        pts.append(pt)
                    for it in range(N_IT):
                        blk, itl = divmod(it, ISH // 128)
                        wt = wmov.tile([128, OT], F16, tag="wmov")
                        nc.sync.dma_start(
                            wt[:],
                            wT_o[h][blk, bass.ts(itl, IT), bass.ts(otl, OT)])
                        q, itq = divmod(it, N_IT // XQ)
                        for bt in range(N_BT):
                            nc.tensor.matmul(
                                pts[bt][:],
                                xthi[q][:, bass.ds(itq * B + bt * BT, BT)],
                                wt[:],
                                start=(it == 0), stop=(it == N_IT - 1))
                    for bt in range(N_BT):
                        st = drain.tile([128, OT], F32, tag="drain")
                        nc.vector.tensor_copy(st[:], pts[bt][:])
                        nc.scalar.dma_start(
                            y[bass.ts(bt, BT), bass.ts(ot, OT)], st[:])

    nc.finalize()
    return nc


def _get_nc():
    if "nc" not in _CACHED:
        _CACHED["nc"] = _build()
    return _CACHED["nc"]


def build_nc(repeat=1, **kw):
    return _build(repeat=repeat, **kw)


def run(x, weight, **run_kwargs):
    nc = _get_nc()
    x = np.ascontiguousarray(x, dtype=np.float32)
    weight = np.ascontiguousarray(weight, dtype=np.float32)
    in_maps = [
        {"x": x[c * B:(c + 1) * B],
         "wsh": np.ascontiguousarray(weight[:, c * ISH:(c + 1) * ISH])}
        for c in range(N_CORES)
    ]
    res = run_bass_kernel_spmd(nc, in_maps, list(range(N_CORES)), **run_kwargs)
    out = np.concatenate([res.results[c]["y"] for c in range(N_CORES)], axis=0)
    return out, res


def kernel(x, weight):
    out, _ = run(x, weight)
    return out
